# revision 2
# baseline (speedup 1.0000x reference)
"""Trainium2 Bass kernel for nn_DiffDelRNN (GRU + time-varying fractional delay line).

Strategy (8 NeuronCores, pure data parallelism over batch N=32 -> 4 seqs/core):
  * GRU (H=8, T=65536, sequential): each sequence is split into F=512 chunks of
    L=128 steps; chunks run in parallel across SBUF free-dim columns with a
    W=32-step warmup (GRU state forgets in ~32 steps; validated to fp32 noise).
    Per pipeline step one PE matmul pair computes all gate pre-activations for
    4 seqs x 512 chunks, ACT does sigmoid/tanh, DVE/Pool do the gate algebra.
  * Output projection (w_out) folded in as a third tiny matmul per step.
  * Delay line: y[t] = w0*xpad[t+k0] + w1*xpad[t+k0+1] with k0=floor(10000-dt);
    implemented as an indirect-DMA pair-gather (8B descriptors) from a DRAM
    xpad = [buffer | pre_d] staging tensor, then 3 DVE ops.

Self-contained: hardcodes all shapes; host-side prep is numpy only.
"""

import os

import numpy as np

N, C, T, H = 32, 1, 65536, 8
MAXD = 10000
NCORES = 8
GPC = 4              # sequences (groups) per core
F = 512              # chunks per sequence == pipeline width (columns)
L = T // F           # 128 steps per chunk
W = 32               # warmup steps (validated: fp32 noise floor)
S = L + W            # 160 pipeline steps
XB = 4               # x-stage DMA block, steps
PB = 8               # pred writeout block, steps
TS = 2048            # delay-phase timesteps per partition (4*32*2048 = 4*T)
XPAD = MAXD + T + 8  # per-seq padded length (tail slots only read with w1==0)
WCHUNK = 4092        # elems of real window data per gather sub-window

MM_DT = os.environ.get("KBASS_MM_DT", "float32")  # float32 | float32r


def _build_host_tensors(x, w_ih, w_hh, b_ih, b_hh, w_out):
    """Per-core numpy inputs. x: (N, T) float32."""
    f32 = np.float32
    # --- permuted stationary matrices -------------------------------------
    # mm output rows: 0:32 Z | 32:64 R | 64:96 B (h-part of n) | 96:128 A (x-part)
    # within a block: row g*8+i (seq g, gate unit i); K rows of h: k = g*8+j.
    lhsT1 = np.zeros((32, 128), f32)
    lhsT2 = np.zeros((5, 128), f32)
    lhsTp = np.zeros((32, 4), f32)
    for g in range(GPC):
        for i in range(H):
            for j in range(H):
                lhsT1[g * 8 + j, 0 + g * 8 + i] = w_hh[8 + i, j]
                lhsT1[g * 8 + j, 32 + g * 8 + i] = w_hh[i, j]
                lhsT1[g * 8 + j, 64 + g * 8 + i] = w_hh[16 + i, j]
            lhsT2[g, 0 + g * 8 + i] = w_ih[8 + i, 0]
            lhsT2[g, 32 + g * 8 + i] = w_ih[i, 0]
            lhsT2[g, 96 + g * 8 + i] = w_ih[16 + i, 0]
            lhsT2[4, 0 + g * 8 + i] = b_ih[8 + i] + b_hh[8 + i]
            lhsT2[4, 32 + g * 8 + i] = b_ih[i] + b_hh[i]
            lhsT2[4, 64 + g * 8 + i] = b_hh[16 + i]
            lhsT2[4, 96 + g * 8 + i] = b_ih[16 + i]
        lhsTp[g * 8:(g + 1) * 8, g] = w_out[0, :]

    # --- per-core x staging: xr[g, s, f] = x[gc, f*L + s - W] (0 if t<0) --
    xr_cores = []
    for c in range(NCORES):
        xs = x[c * GPC:(c + 1) * GPC]  # (4, T)
        xr = np.zeros((5, S, F), f32)
        xr[4] = 1.0  # ones row (bias)
        # t = f*L + s - W ; valid when >= 0
        tgrid = (np.arange(F)[None, :] * L + np.arange(S)[:, None] - W)  # (S, F)
        valid = tgrid >= 0
        tcl = np.clip(tgrid, 0, T - 1)
        for g in range(GPC):
            xr[g] = np.where(valid, xs[g][tcl], 0.0)
        xr_cores.append(np.ascontiguousarray(xr.reshape(5, S * F)))

    return lhsT1, lhsT2, lhsTp, xr_cores


def _build_delay_tensors(del_traj):
    """Host-side delay-line gather plan. del_traj: (N, T) float32.

    Per core: window[p] = xpad[unit_p, tile*TS : tile*TS + WLEN]; local gather
    index jl[p, u] = u + floor(MAXD - dt). indirect_copy consumes one shared
    index stream per gpsimd core (16 partitions), element i of core c's stream
    at [16c + i%16, i//16]; call k routes unit 16c+k on core c.
    """
    f32 = np.float32
    wridx_cores, frac_cores = [], []
    for c in range(NCORES):
        dts = del_traj[c * GPC:(c + 1) * GPC]              # (4, T)
        d = dts.reshape(GPC, 32, TS).reshape(128, TS)      # natural [p, u]
        pp = (np.float32(MAXD) - d).astype(f32)
        k0 = np.floor(pp).astype(f32)
        frac = (pp - k0).astype(f32)
        jl = (k0.astype(np.int64) + np.arange(TS)[None, :])
        # 3 sub-windows of <=4096 elems (walrus IndirectCopy data limit);
        # out-of-range indices hit sentinel zero cols 4094/4095.
        wr = np.zeros((128, 3 * TS), np.uint16)
        cw = TS // 16
        i = np.arange(TS)
        for w in range(3):
            b = w * WCHUNK
            sw = np.where((jl >= b) & (jl < b + WCHUNK), jl - b, 4094)
            sw = sw.astype(np.uint16)
            for k in range(16):
                for cc in range(8):
                    wr[16 * cc + (i % 16), w * TS + k * cw + i // 16] = \
                        sw[16 * cc + k, i]
        wridx_cores.append(wr)
        frac_cores.append(frac)
    return wridx_cores, frac_cores


def _build_program():
    import concourse.bacc as bacc
    import concourse.bass as bass
    import concourse.mybir as mybir
    import concourse.tile as tile
    from concourse.alu_op_type import AluOpType

    f32 = mybir.dt.float32
    mmdt = getattr(mybir.dt, MM_DT)
    ACT = mybir.ActivationFunctionType

    nc = bacc.Bacc("TRN2", target_bir_lowering=False, debug=False)

    # I/O ------------------------------------------------------------------
    xr_t = nc.dram_tensor("xr", [5, S * F], f32, kind="ExternalInput")
    del_t = nc.dram_tensor("del", [GPC, T], f32, kind="ExternalInput")
    buf_t = nc.dram_tensor("buf", [GPC, MAXD], f32, kind="ExternalInput")
    lhsT1_t = nc.dram_tensor("lhsT1", [32, 128], f32, kind="ExternalInput")
    lhsT2_t = nc.dram_tensor("lhsT2", [5, 128], f32, kind="ExternalInput")
    lhsTp_t = nc.dram_tensor("lhsTp", [32, 4], f32, kind="ExternalInput")
    wridx_t = nc.dram_tensor("wridx", [128, 3 * TS], mybir.dt.uint16,
                             kind="ExternalInput")
    frac_t = nc.dram_tensor("frac", [128, TS], f32, kind="ExternalInput")
    y_t = nc.dram_tensor("y", [GPC, T], f32, kind="ExternalOutput")
    pred_t = nc.dram_tensor("pred", [GPC, T], f32, kind="ExternalOutput")

    with tile.TileContext(nc) as tc:
        import contextlib
        est = contextlib.ExitStack()
        gru_est = contextlib.ExitStack()
        with est:
            # ---- long-lived pools --------------------------------------
            wpool = est.enter_context(tc.tile_pool(name="wpool", bufs=1))
            idxp = est.enter_context(tc.tile_pool(name="idxp", bufs=1))
            iscr = est.enter_context(tc.tile_pool(name="iscr", bufs=4))
            ipool = est.enter_context(tc.tile_pool(name="ipool", bufs=1))
            dramp = est.enter_context(tc.tile_pool(name="dramp", bufs=1, space="DRAM"))
            # ---- GRU-phase pools (closed before delay phase) -----------
            hpool = gru_est.enter_context(tc.tile_pool(name="hpool", bufs=3))
            xpool = gru_est.enter_context(tc.tile_pool(name="xpool", bufs=2))
            pspool = gru_est.enter_context(tc.tile_pool(name="pspool", bufs=2, space="PSUM"))
            prepool = gru_est.enter_context(tc.tile_pool(name="prepool", bufs=2, space="PSUM"))
            rzpool = gru_est.enter_context(tc.tile_pool(name="rzpool", bufs=2))
            tpool = gru_est.enter_context(tc.tile_pool(name="tpool", bufs=2))
            upool = gru_est.enter_context(tc.tile_pool(name="upool", bufs=2))
            npool = gru_est.enter_context(tc.tile_pool(name="npool", bufs=2))
            zhpool = gru_est.enter_context(tc.tile_pool(name="zhpool", bufs=2))
            qpool = gru_est.enter_context(tc.tile_pool(name="qpool", bufs=2))
            pstg = gru_est.enter_context(tc.tile_pool(name="pstg", bufs=2))

            # ---- static weights in SBUF --------------------------------
            w1 = wpool.tile([32, 128], f32)
            w2 = wpool.tile([5, 128], f32)
            wp = wpool.tile([32, 4], f32)
            nc.sync.dma_start(w1[:], lhsT1_t[:])
            nc.sync.dma_start(w2[:], lhsT2_t[:])
            nc.sync.dma_start(wp[:], lhsTp_t[:])

            def mmcast(ap):
                return ap.bitcast(mmdt) if mmdt != f32 else ap

            # ---- xpad staging tensor in DRAM ---------------------------
            xpad = dramp.tile([GPC, XPAD], f32)
            nc.sync.dma_start(xpad[:, 0:MAXD], buf_t[:])
            zpad = wpool.tile([GPC, 2], f32)
            nc.vector.memset(zpad[:], 0.0)
            nc.sync.dma_start(xpad[:, XPAD - 2:XPAD], zpad[:])

            # ---- delay-line gather inputs (host-computed) --------------
            wridx = idxp.tile([128, 3 * TS], mybir.dt.uint16, tag="wridx")
            nc.sync.dma_start(wridx[:], wridx_t[:])
            frac = idxp.tile([128, TS], f32, tag="frac")
            nc.sync.dma_start(frac[:], frac_t[:])

            # ---- GRU pipeline ------------------------------------------
            h = hpool.tile([32, F], f32, tag="h")
            nc.vector.memset(h[:], 0.0)
            xstage = None
            pstage = None
            for s in range(S):
                if s % XB == 0:
                    xstage = xpool.tile([5, XB * F], f32, tag="xstage")
                    nc.sync.dma_start(
                        xstage[:], xr_t[:, s * F:(s + XB) * F])
                xs = xstage[:, (s % XB) * F:((s % XB) + 1) * F]

                psum = pspool.tile([128, F], f32, tag="ps")
                nc.tensor.matmul(psum[:], mmcast(w1[:]), mmcast(h[:]),
                                 start=True, stop=False)
                nc.tensor.matmul(psum[:], mmcast(w2[:]), mmcast(xs),
                                 start=False, stop=True)

                rz = rzpool.tile([64, F], f32, tag="rz")
                nc.scalar.activation(rz[:], psum[0:64, :], ACT.Sigmoid)
                t1 = tpool.tile([32, F], f32, tag="t1")
                nc.vector.tensor_tensor(out=t1[:], in0=rz[32:64, :],
                                        in1=psum[64:96, :], op=AluOpType.mult)
                t2 = upool.tile([32, F], f32, tag="t2")
                nc.vector.tensor_tensor(out=t2[:], in0=t1[:],
                                        in1=psum[96:128, :], op=AluOpType.add)
                nn = npool.tile([32, F], f32, tag="nn")
                nc.scalar.activation(nn[:], t2[:], ACT.Tanh)
                zh = zhpool.tile([32, F], f32, tag="zh")
                nc.gpsimd.tensor_tensor(out=zh[:], in0=rz[0:32, :], in1=h[:],
                                        op=AluOpType.mult)
                q = qpool.tile([32, F], f32, tag="q")
                nc.vector.scalar_tensor_tensor(
                    out=q[:], in0=rz[0:32, :], scalar=1.0, in1=nn[:],
                    op0=AluOpType.subtract, op1=AluOpType.mult,
                )
                hn = hpool.tile([32, F], f32, tag="h")
                nc.vector.tensor_tensor(out=hn[:], in0=zh[:], in1=q[:],
                                        op=AluOpType.subtract)
                if s == W - 1:
                    # chunk f=0 must start from exact h=0 at t=0
                    nc.vector.memset(hn[:, 0:1], 0.0)
                h = hn

                if s >= W:
                    u = s - W
                    pps = prepool.tile([4, F], f32, tag="pps")
                    nc.tensor.matmul(pps[:], mmcast(wp[:]), mmcast(h[:]),
                                     start=True, stop=True)
                    if u % PB == 0:
                        pstage = pstg.tile([4, F * PB], f32, tag="pstage")
                    uu = u % PB
                    # predstage col = f*PB + uu
                    nc.scalar.activation(
                        pstage[:, uu:uu + (F - 1) * PB + 1:PB], pps[:],
                        ACT.Copy)
                    if uu == PB - 1:
                        u0 = u - (PB - 1)
                        # DRAM addr = u0 + uu2 + L*f, SBUF walks (f, uu2)
                        from concourse.ap import AP as _AP
                        pap = pred_t[:]
                        xap = xpad[:]
                        dst1 = _AP(pap.tensor, pap.offset + u0,
                                   [[T, GPC], [L, F], [1, PB]])
                        dst2 = _AP(xap.tensor, xap.offset + MAXD + u0,
                                   [[XPAD, GPC], [L, F], [1, PB]])
                        src = pstage[:].rearrange("g (f uu) -> g f uu", uu=PB)
                        nc.sync.dma_start(dst1, src)
                        nc.sync.dma_start(dst2, src)

            # ---- close GRU-phase pools, then the delay tail ------------
            gru_est.close()
            tc.strict_bb_all_engine_barrier()

            # ---- delay line: gpsimd indirect_copy pair-gather ----------
            # window[p] = xpad[unit_p, tile*TS : tile*TS + WLEN]  (48KB/part)
            from concourse.ap import AP as _AP
            WLEN = TS + MAXD + 2
            dpool = est.enter_context(tc.tile_pool(name="dpool", bufs=1))
            qpool2 = est.enter_context(tc.tile_pool(name="qpool2", bufs=2))
            from concourse.tile import add_dep_helper as _adh
            win_dmas = []
            wins = []
            for w in range(3):
                wlen = min(WCHUNK + 2, TS + MAXD + 2 - w * WCHUNK)
                winw = dpool.tile([128, 4096], f32, tag=f"win{w}")
                xap = xpad[:]
                wsrc = _AP(xap.tensor, xap.offset + w * WCHUNK,
                           [[XPAD, GPC], [TS, 32], [1, wlen]])
                d1 = nc.sync.dma_start(winw[:, 0:wlen], wsrc)
                zw = nc.vector.memset(winw[:, 4094:4096], 0.0)
                wins.append(winw)
                win_dmas.append((d1, zw))

            pairs = dramp.tile([128, 2 * TS], f32)
            CW = TS // 16  # wridx columns per call-unit
            NIB = 512      # indices per gather sub-call (ISA limit)
            nb = TS // NIB
            cb = NIB // 16
            spool2 = est.enter_context(tc.tile_pool(name="spool2", bufs=2))
            for k in range(16):
                for b in range(nb):
                    scrs = []
                    for w in range(3):
                        scr = spool2.tile([128, NIB, 2], f32, tag=f"sc{w}")
                        c0 = w * TS + CW * k + cb * b
                        gi = nc.gpsimd.add_instruction(
                            mybir.InstIndirectCopy(
                                name=f"I-{nc.next_id()}",
                                ins=[nc.gpsimd.lower_ap(wins[w][:]),
                                     nc.gpsimd.lower_ap(wridx[:, c0:c0 + cb])],
                                outs=[nc.gpsimd.lower_ap(scr[:])],
                                num_valid_indices=NIB,
                            ))
                        _adh(gi.ins, win_dmas[w][0].ins, sync=True, reason="g")
                        _adh(gi.ins, win_dmas[w][1].ins, sync=True, reason="g")
                        scrs.append((scr, gi))
                    s01 = qpool2.tile([128, 2 * NIB], f32, tag="s01")
                    a1 = nc.vector.tensor_tensor(
                        out=s01[:],
                        in0=scrs[0][0][:].rearrange("p u e -> p (u e)"),
                        in1=scrs[1][0][:].rearrange("p u e -> p (u e)"),
                        op=AluOpType.add)
                    ssum = qpool2.tile([128, 2 * NIB], f32, tag="ssum")
                    a2 = nc.vector.tensor_tensor(
                        out=ssum[:], in0=s01[:],
                        in1=scrs[2][0][:].rearrange("p u e -> p (u e)"),
                        op=AluOpType.add)
                    for _, gi in scrs:
                        _adh(a1.ins, gi.ins, sync=True, reason="s")
                        _adh(a2.ins, gi.ins, sync=True, reason="s")
                    pap = pairs[:]
                    dst = _AP(pap.tensor,
                              pap.offset + k * 2 * TS + b * 2 * NIB,
                              [[16 * 2 * TS, 8], [1, 2 * NIB]])
                    ed = nc.sync.dma_start(dst, ssum[:][k:k + 113:16, :])
                    _adh(ed.ins, a2.ins, sync=True, reason="e")

            prld = idxp.tile([128, 2 * TS], f32, tag="prld")
            nc.sync.dma_start(prld[:], pairs[:])
            y0 = prld[:].rearrange("p (u e) -> p u e", e=2)[:, :, 0:1]\
                .rearrange("p u e -> p (u e)")
            y1 = prld[:].rearrange("p (u e) -> p u e", e=2)[:, :, 1:2]\
                .rearrange("p u e -> p (u e)")
            q0 = idxp.tile([128, TS], f32, tag="q0")
            nc.vector.scalar_tensor_tensor(
                out=q0[:], in0=frac[:], scalar=1.0, in1=y0,
                op0=AluOpType.subtract, op1=AluOpType.mult,
            )
            q1 = idxp.tile([128, TS], f32, tag="q1")
            nc.vector.tensor_tensor(out=q1[:], in0=frac[:], in1=y1,
                                    op=AluOpType.mult)
            yt = idxp.tile([128, TS], f32, tag="yt")
            nc.vector.tensor_tensor(out=yt[:], in0=q1[:], in1=q0[:],
                                    op=AluOpType.subtract)
            nc.sync.dma_start(
                y_t[:].rearrange("g (a u) -> (g a) u", a=32), yt[:])

    nc.compile()
    return nc


_NC_CACHE = {}


def kernel(x, del_traj, buffer, w_ih, w_hh, b_ih, b_hh, w_out):
    from concourse.bass_utils import run_bass_kernel_spmd

    x = np.asarray(x, np.float32).reshape(N, T)
    del_traj = np.asarray(del_traj, np.float32).reshape(N, T)
    buffer = np.asarray(buffer, np.float32).reshape(N, MAXD)
    w_ih = np.asarray(w_ih, np.float32)
    w_hh = np.asarray(w_hh, np.float32)
    b_ih = np.asarray(b_ih, np.float32)
    b_hh = np.asarray(b_hh, np.float32)
    w_out = np.asarray(w_out, np.float32)

    lhsT1, lhsT2, lhsTp, xr_cores = _build_host_tensors(
        x, w_ih, w_hh, b_ih, b_hh, w_out)
    wridx_cores, frac_cores = _build_delay_tensors(del_traj)

    if "nc" not in _NC_CACHE:
        _NC_CACHE["nc"] = _build_program()
    nc = _NC_CACHE["nc"]

    in_maps = []
    for c in range(NCORES):
        sl = slice(c * GPC, (c + 1) * GPC)
        in_maps.append({
            "xr": xr_cores[c],
            "del": np.ascontiguousarray(del_traj[sl]),
            "buf": np.ascontiguousarray(buffer[sl]),
            "lhsT1": lhsT1, "lhsT2": lhsT2, "lhsTp": lhsTp,
            "wridx": wridx_cores[c], "frac": frac_cores[c],
        })

    trace = bool(int(os.environ.get("KBASS_TRACE", "0")))
    tmpdir = os.environ.get("KBASS_TMPDIR") or None
    try:
        res = run_bass_kernel_spmd(nc, in_maps, list(range(NCORES)),
                                   trace=trace, tmpdir=tmpdir)
    except ModuleNotFoundError:
        # NTFF profile hook unavailable in this container; run untraced.
        res = run_bass_kernel_spmd(nc, in_maps, list(range(NCORES)))
    if res.exec_time_ns is not None:
        print(f"HW exec time: {res.exec_time_ns} ns")
    if res.instructions_and_trace is not None:
        print(f"trace path: {res.instructions_and_trace[1]}")

    y = np.zeros((N, 1, T), np.float32)
    pred = np.zeros((N, 1, T), np.float32)
    for c in range(NCORES):
        sl = slice(c * GPC, (c + 1) * GPC)
        y[sl, 0, :] = res.results[c]["y"]
        pred[sl, 0, :] = res.results[c]["pred"]
    return (y, pred)



# revision 15
# speedup vs baseline: 2.2517x; 2.2517x over previous
"""Trainium2 Bass kernel for nn_DiffDelRNN (GRU + time-varying fractional delay).

v2 design (8 NeuronCores, data-parallel over batch N=32 -> 4 seqs/core):

GRU phase: T=65536 is split into F=2048 chunks of L=32 steps, run as 2
"pairs"; each pair partition-merges 2 batches of 512 chunks so every gate op
covers 128 (or 64) partitions at 512 columns. W=16 warmup steps per chunk
(validated: rel err ~2e-5). Per pair per step: one fp32r matmul computes all
sigmoid pre-activations [z_A z_B r_A r_B] (K=73: 64 h rows + 8 x rows + ones),
a second computes [ghn_A ghn_B gin_A gin_B]; ACT does sigmoid/tanh; DVE/Pool
do the 5-op gate algebra. Pred (w_out.h) is a third tiny matmul into a
2-bank psum tile, copied out every 2 steps and DMAed to a u-major DRAM
staging layout (contiguous 2KB runs, no transpose needed on device).

Delay phase: y[t] = (1-frac)*xpad[j] + frac*xpad[j+1], j = floor(1e4-dt)+t.
The staging tensor stg[g] holds [buffer | pred] in u-major order
(addr = u*2361 + (t+10016)//32); windows win[p=(g,a), 12064] are loaded with
an affine AP and gpsimd ap_gather (full-span window, d=1, 2048 idx/call,
host-permuted indices) gathers both taps; 32 calls x (8 valid rows each) are
re-assembled by partition-strided SBUF DMAs, combined by 3 DVE ops.

Self-contained: hardcodes all shapes; host-side prep is numpy only.
"""

import os

import ml_dtypes
import numpy as np

BF16 = ml_dtypes.bfloat16

N, C, T, H = 32, 1, 65536, 8
MAXD = 10000
NCORES = 8
GPC = 4               # sequences per core
NPAIR = 2             # chunk-pair pipelines per core
FB = 512              # chunk columns per batch (= matmul free dim)
FTOT = 2048           # total chunks per core (= NPAIR * 2 * FB)
L = T // FTOT         # 32 timesteps per chunk
W = 16                # warmup steps (validated ~2e-5 rel err)
S = L + W             # pipeline steps per pair
TS = 2048             # delay-phase timesteps per partition row
BUFC = 313            # buffer chunks of 32 (313*32 = 10016 >= MAXD+pad)
ROWF = BUFC + 64      # 377 window f-entries per u
WINC = 32 * ROWF      # 12064 window cols per partition
STGW = 32 * (BUFC + FTOT)   # 75552 per-seq staging width

MM_DT = os.environ.get("KBASS_MM_DT", "float32r")
NIDX = 2048           # ap_gather indices per call


def _build_gru_host(x, w_ih, w_hh, b_ih, b_hh, w_out):
    """lhsT matrices + per-core xr staging. x: (N, T) f32."""
    f32 = np.float32
    lhsTs = np.zeros((73, 128), f32)   # -> [z_A z_B r_A r_B]
    lhsTn = np.zeros((73, 128), f32)   # -> [ghn_A ghn_B gin_A gin_B]
    lhsTp = np.zeros((64, 8), f32)     # -> pred [A(seq0..3) B(seq0..3)]
    for b in range(2):                 # batch A/B within a pair
        for g in range(GPC):
            for i in range(H):
                m = b * 32 + g * 8 + i
                for j in range(H):
                    k = b * 32 + g * 8 + j
                    lhsTs[k, m] = w_hh[H + i, j]          # z
                    lhsTs[k, 64 + m] = w_hh[i, j]          # r
                    lhsTn[k, m] = w_hh[2 * H + i, j]       # ghn
                kx = 64 + b * 4 + g
                lhsTs[kx, m] = w_ih[H + i, 0]
                lhsTs[kx, 64 + m] = w_ih[i, 0]
                lhsTn[kx, 64 + m] = w_ih[2 * H + i, 0]     # gin
                lhsTs[72, m] = b_ih[H + i] + b_hh[H + i]
                lhsTs[72, 64 + m] = b_ih[i] + b_hh[i]
                lhsTn[72, m] = b_hh[2 * H + i]
                lhsTn[72, 64 + m] = b_ih[2 * H + i]
                lhsTp[b * 32 + g * 8 + i, b * 4 + g] = w_out[0, i]

    # xr[pair, row, s*FB + col]; rows 0:4 x_A(seq), 4:8 x_B(seq), 8 ones
    xr_cores = []
    for c in range(NCORES):
        xs = x[c * GPC:(c + 1) * GPC]                  # (4, T)
        xr = np.zeros((NPAIR, 9, S * FB), f32)
        xr[:, 8, :] = 1.0
        col = np.arange(FB)
        for pair in range(NPAIR):
            for b in range(2):
                f = (pair * 2 + b) * FB + col          # (FB,)
                for s in range(S):
                    t = f * L + s - W
                    v = np.where(t >= 0, xs[:, np.clip(t, 0, T - 1)], 0.0)
                    xr[pair, b * 4:(b + 1) * 4, s * FB:(s + 1) * FB] = v
        xr_cores.append(np.ascontiguousarray(
            xr.reshape(NPAIR * 9, S * FB).astype(BF16)))
    return lhsTs.astype(BF16), lhsTn.astype(BF16), lhsTp.astype(BF16), \
        xr_cores


def _build_delay_host(del_traj, buffer):
    """Gather indices (host-permuted u-major), fracs, staged buffer."""
    f32 = np.float32
    wr0_c, wr1_c, frac_c, bufstg_c = [], [], [], []
    arow = np.repeat(np.arange(GPC * 32) % 32, TS).reshape(128, TS)  # a per row
    tpos = (arow * TS + np.arange(TS)[None, :]).astype(np.int64)     # abs t
    for c in range(NCORES):
        dts = del_traj[c * GPC:(c + 1) * GPC].reshape(GPC, 32, TS)
        d = dts.reshape(128, TS)
        p = (np.float32(MAXD) - d).astype(f32)
        k0 = np.floor(p)
        frac = (p - k0).astype(f32)
        k0i = k0.astype(np.int64)
        k1i = np.minimum(k0i + 1, MAXD)
        t0 = tpos + k0i - MAXD                     # tap t-time in [-9999, t]
        t1 = tpos + k1i - MAXD
        wrs = []
        for tt in (t0, t1):
            jj = tt + 32 * BUFC                    # >= 17
            jloc = (jj % 32) * ROWF + jj // 32 - 64 * arow
            assert jloc.min() >= 0 and jloc.max() < WINC, (
                jloc.min(), jloc.max())
            # wrap into per-call streams: call k uses cols [k*128,(k+1)*128);
            # element u of core c's stream sits at [16c + u%16, k*128 + u//16]
            wr = np.zeros((128, 16 * (NIDX // 16)), np.int16)
            u = np.arange(TS)
            for prow in range(128):
                cc, k = prow // 16, prow % 16
                wr[16 * cc + (u % 16), k * (NIDX // 16) + u // 16] = \
                    jloc[prow].astype(np.int16)
            wrs.append(wr)
        wr0_c.append(wrs[0]); wr1_c.append(wrs[1]); frac_c.append(frac)
        # buffer staged u-major: bufstg[g, u*BUFC + f'] = buffer[g, t+MAXD],
        # t = f'*32 + u - 32*BUFC  (t < -MAXD -> 0 pad)
        bs = np.zeros((GPC, 32, BUFC), f32)
        bufc = buffer[c * GPC:(c + 1) * GPC]
        fidx = np.arange(BUFC)
        for u in range(32):
            t = fidx * 32 + u - 32 * BUFC
            valid = t >= -MAXD
            bs[:, u, :] = np.where(valid[None, :],
                                   bufc[:, np.clip(t + MAXD, 0, MAXD - 1)], 0.0)
        bufstg_c.append(np.ascontiguousarray(bs.reshape(GPC, 32 * BUFC)))
    return wr0_c, wr1_c, frac_c, bufstg_c


def _build_program():
    import concourse.bacc as bacc
    import concourse.mybir as mybir
    import concourse.tile as tile
    from concourse.alu_op_type import AluOpType
    from concourse.ap import AP as _AP

    f32 = mybir.dt.float32
    bf16 = mybir.dt.bfloat16
    i16 = mybir.dt.int16
    ACT = mybir.ActivationFunctionType

    nc = bacc.Bacc("TRN2", target_bir_lowering=False, debug=False)

    # ---- I/O -------------------------------------------------------------
    xr_t = nc.dram_tensor("xr", [NPAIR * 9, S * FB], bf16,
                          kind="ExternalInput")
    lhsTs_t = nc.dram_tensor("lhsTs", [73, 128], bf16, kind="ExternalInput")
    lhsTn_t = nc.dram_tensor("lhsTn", [73, 128], bf16, kind="ExternalInput")
    lhsTp_t = nc.dram_tensor("lhsTp", [64, 8], bf16, kind="ExternalInput")
    bufstg_t = nc.dram_tensor("bufstg", [GPC, 32 * BUFC], f32,
                              kind="ExternalInput")
    wr0_t = nc.dram_tensor("wr0", [128, NIDX], i16, kind="ExternalInput")
    wr1_t = nc.dram_tensor("wr1", [128, NIDX], i16, kind="ExternalInput")
    frac_t = nc.dram_tensor("frac", [128, TS], f32, kind="ExternalInput")
    stg_t = nc.dram_tensor("stg", [GPC, STGW], f32, kind="ExternalOutput")
    y_t = nc.dram_tensor("y", [GPC, T], f32, kind="ExternalOutput")

    with tile.TileContext(nc) as tc:
        import contextlib
        est = contextlib.ExitStack()
        gru_est = contextlib.ExitStack()
        with est:
            wpool = est.enter_context(tc.tile_pool(name="wpool", bufs=1))
            idxp = est.enter_context(tc.tile_pool(name="idxp", bufs=1))

            lts = wpool.tile([73, 128], bf16)
            ltn = wpool.tile([73, 128], bf16)
            ltp = wpool.tile([64, 8], bf16)
            nc.sync.dma_start(lts[:], lhsTs_t[:])
            nc.sync.dma_start(ltn[:], lhsTn_t[:])
            nc.sync.dma_start(ltp[:], lhsTp_t[:])

            wr0 = idxp.tile([128, NIDX], i16, tag="wr0")
            wr1 = idxp.tile([128, NIDX], i16, tag="wr1")
            frac = idxp.tile([128, TS], f32, tag="frac")
            nc.sync.dma_start(wr0[:], wr0_t[:])
            nc.sync.dma_start(wr1[:], wr1_t[:])
            nc.sync.dma_start(frac[:], frac_t[:])

            # staged buffer -> stg[g, u*2361 + f'], f' in [0, BUFC)
            sap = stg_t[:]
            dstb = _AP(sap.tensor, sap.offset,
                       [[STGW, GPC], [BUFC + FTOT, 32], [1, BUFC]])
            nc.sync.dma_start(dstb, bufstg_t[:])

            # ---- GRU phase ---------------------------------------------
            hxp = [gru_est.enter_context(
                tc.tile_pool(name=f"hx{p}", bufs=2)) for p in range(NPAIR)]
            psS = [gru_est.enter_context(
                tc.tile_pool(name=f"psS{p}", bufs=1, space="PSUM"))
                for p in range(NPAIR)]
            psN = [gru_est.enter_context(
                tc.tile_pool(name=f"psN{p}", bufs=1, space="PSUM"))
                for p in range(NPAIR)]
            psP = [gru_est.enter_context(
                tc.tile_pool(name=f"psP{p}", bufs=1, space="PSUM"))
                for p in range(NPAIR)]
            rzp = gru_est.enter_context(tc.tile_pool(name="rzp", bufs=2))
            up = gru_est.enter_context(tc.tile_pool(name="up", bufs=2))
            tp = gru_est.enter_context(tc.tile_pool(name="tp", bufs=2))
            np_ = gru_est.enter_context(tc.tile_pool(name="np", bufs=2))
            zp = gru_est.enter_context(tc.tile_pool(name="zp", bufs=2))
            qp = gru_est.enter_context(tc.tile_pool(name="qp", bufs=2))
            stp = gru_est.enter_context(tc.tile_pool(name="stp", bufs=1))

            pstage = []
            for p in range(NPAIR):
                pst = stp.tile([8, (L // 2) * 1024], f32, tag=f"pst{p}",
                               name=f"pst{p}")
                pstage.append(pst)

            hx = []
            for p in range(NPAIR):
                t0 = hxp[p].tile([73, FB], bf16, tag=f"hx{p}")
                nc.vector.memset(t0[0:64, :], 0.0)
                nc.sync.dma_start(t0[64:73, :],
                                  xr_t[p * 9:(p + 1) * 9, 0:FB])
                hx.append(t0)

            ppred = [None] * NPAIR
            for s in range(S):
                for p in range(NPAIR):
                    cur = hx[p]
                    nxt = hxp[p].tile([73, FB], bf16, tag=f"hx{p}")
                    if s + 1 < S:
                        nc.sync.dma_start(
                            nxt[64:73, :],
                            xr_t[p * 9:(p + 1) * 9,
                                 (s + 1) * FB:(s + 2) * FB])
                    ps = psS[p].tile([128, FB], f32, tag=f"psS{p}")
                    nc.tensor.matmul(ps[:], lts[:], cur[:],
                                     start=True, stop=True)
                    pn = psN[p].tile([128, FB], f32, tag=f"psN{p}")
                    nc.tensor.matmul(pn[:], ltn[:], cur[:],
                                     start=True, stop=True)
                    rz = rzp.tile([128, FB], bf16, tag=f"rz{p}")
                    nc.scalar.activation(rz[:], ps[:], ACT.Sigmoid)
                    u = up.tile([64, FB], bf16, tag=f"u{p}")
                    nc.vector.tensor_tensor(out=u[:], in0=rz[64:128, :],
                                            in1=pn[0:64, :],
                                            op=AluOpType.mult)
                    t2 = tp.tile([64, FB], bf16, tag=f"t2{p}")
                    nc.vector.tensor_tensor(out=t2[:], in0=u[:],
                                            in1=pn[64:128, :],
                                            op=AluOpType.add)
                    nn = np_.tile([64, FB], bf16, tag=f"nn{p}")
                    nc.scalar.activation(nn[:], t2[:], ACT.Tanh)
                    zh = zp.tile([64, FB], bf16, tag=f"zh{p}")
                    nc.gpsimd.tensor_tensor(out=zh[:], in0=rz[0:64, :],
                                            in1=cur[0:64, :],
                                            op=AluOpType.mult)
                    q = qp.tile([64, FB], bf16, tag=f"q{p}")
                    eng = nc.vector
                    eng.scalar_tensor_tensor(
                        out=q[:], in0=rz[0:64, :], scalar=1.0, in1=nn[:],
                        op0=AluOpType.subtract, op1=AluOpType.mult)
                    # q = (z - 1) * n, so h' = z*h + (1-z)*n = zh - q
                    nc.vector.tensor_tensor(out=nxt[0:64, :], in0=zh[:],
                                            in1=q[:], op=AluOpType.subtract)
                    if s == W - 1 and p == 0:
                        nc.vector.memset(nxt[0:32, 0:1], 0.0)
                    if s >= W:
                        uu = s - W
                        if uu % 2 == 0:
                            ppred[p] = psP[p].tile(
                                [8, 1024], f32, tag=f"psP{p}",
                                name=f"psP{p}")
                        pp = ppred[p]
                        nc.tensor.matmul(
                            pp[:, (uu % 2) * FB:(uu % 2 + 1) * FB],
                            ltp[:], nxt[0:64, :],
                            start=True, stop=True)
                        if uu % 2 == 1:
                            nc.scalar.activation(
                                pstage[p][0:8, (uu // 2) * 1024:
                                          (uu // 2 + 1) * 1024],
                                pp[:], ACT.Copy)
                    hx[p] = nxt

            # pstage[p][row=(b g), c*1024 + par*512 + col] ->
            #   stg[g, (2c+par)*2361 + BUFC + (p*2+b)*512 + col]
            for p in range(NPAIR):
                for b in range(2):
                    for par in range(2):
                        src = pstage[p][b * 4:(b + 1) * 4, :]\
                            .rearrange("r (c pe e) -> r c pe e", pe=2, e=512)\
                            [:, :, par, :]
                        dst = _AP(sap.tensor,
                                  sap.offset + BUFC + (p * 2 + b) * 512
                                  + par * (BUFC + FTOT),
                                  [[STGW, GPC],
                                   [2 * (BUFC + FTOT), L // 2], [1, 512]])
                        nc.sync.dma_start(dst, src)

            gru_est.close()
            tc.strict_bb_all_engine_barrier()

            # ---- delay phase -------------------------------------------
            dpool = est.enter_context(tc.tile_pool(name="dpool", bufs=1))
            gpool = est.enter_context(tc.tile_pool(name="gpool", bufs=4))
            ypool = est.enter_context(tc.tile_pool(name="ypool", bufs=1))

            win = dpool.tile([128, WINC], f32)
            for uix in range(32):
                wsrc = _AP(sap.tensor, sap.offset + uix * (BUFC + FTOT),
                           [[STGW, GPC], [64, 32], [1, ROWF]])
                nc.sync.dma_start(win[:, uix * ROWF:(uix + 1) * ROWF], wsrc)

            y0 = ypool.tile([128, TS], f32, tag="y0")
            y1 = ypool.tile([128, TS], f32, tag="y1")
            CW = NIDX // 16
            for k in range(16):
                for tap, (wr, yb) in enumerate(((wr0, y0), (wr1, y1))):
                    g = gpool.tile([128, NIDX], f32, tag=f"g{tap}")
                    nc.gpsimd.ap_gather(
                        g[:], win[:], wr[:, k * CW:(k + 1) * CW],
                        channels=128, num_elems=WINC, d=1, num_idxs=NIDX)
                    nc.sync.dma_start(yb[:][k:k + 113:16, :],
                                      g[:][k:k + 113:16, :])
            dd = ypool.tile([128, TS], f32, tag="dd")
            nc.vector.tensor_tensor(out=dd[:], in0=y1[:], in1=y0[:],
                                    op=AluOpType.subtract)
            mm_ = ypool.tile([128, TS], f32, tag="mm")
            nc.vector.tensor_tensor(out=mm_[:], in0=frac[:], in1=dd[:],
                                    op=AluOpType.mult)
            yt = ypool.tile([128, TS], f32, tag="yt")
            nc.vector.tensor_tensor(out=yt[:], in0=y0[:], in1=mm_[:],
                                    op=AluOpType.add)
            nc.sync.dma_start(
                y_t[:].rearrange("g (a u) -> (g a) u", a=32), yt[:])

    nc.compile()
    return nc


_NC_CACHE = {}


def kernel(x, del_traj, buffer, w_ih, w_hh, b_ih, b_hh, w_out):
    from concourse.bass_utils import run_bass_kernel_spmd

    x = np.asarray(x, np.float32).reshape(N, T)
    del_traj = np.asarray(del_traj, np.float32).reshape(N, T)
    buffer = np.asarray(buffer, np.float32).reshape(N, MAXD)
    w_ih = np.asarray(w_ih, np.float32)
    w_hh = np.asarray(w_hh, np.float32)
    b_ih = np.asarray(b_ih, np.float32)
    b_hh = np.asarray(b_hh, np.float32)
    w_out = np.asarray(w_out, np.float32)

    lhsTs, lhsTn, lhsTp, xr_cores = _build_gru_host(
        x, w_ih, w_hh, b_ih, b_hh, w_out)
    wr0_c, wr1_c, frac_c, bufstg_c = _build_delay_host(del_traj, buffer)

    if "nc" not in _NC_CACHE:
        _NC_CACHE["nc"] = _build_program()
    nc = _NC_CACHE["nc"]

    in_maps = []
    for c in range(NCORES):
        in_maps.append({
            "xr": xr_cores[c],
            "lhsTs": lhsTs, "lhsTn": lhsTn, "lhsTp": lhsTp,
            "bufstg": bufstg_c[c],
            "wr0": wr0_c[c], "wr1": wr1_c[c], "frac": frac_c[c],
        })

    trace = bool(int(os.environ.get("KBASS_TRACE", "0")))
    tmpdir = os.environ.get("KBASS_TMPDIR") or None
    try:
        res = run_bass_kernel_spmd(nc, in_maps, list(range(NCORES)),
                                   trace=trace, tmpdir=tmpdir)
    except ModuleNotFoundError:
        res = run_bass_kernel_spmd(nc, in_maps, list(range(NCORES)))
    if res.exec_time_ns is not None:
        print(f"HW exec time: {res.exec_time_ns} ns")
    if res.instructions_and_trace is not None:
        print(f"trace path: {res.instructions_and_trace[1]}")

    y = np.zeros((N, 1, T), np.float32)
    pred = np.zeros((N, 1, T), np.float32)
    for c in range(NCORES):
        sl = slice(c * GPC, (c + 1) * GPC)
        y[sl, 0, :] = res.results[c]["y"]
        v = res.results[c]["stg"].reshape(GPC, 32, BUFC + FTOT)
        pred[sl, 0, :] = v[:, :, BUFC:].transpose(0, 2, 1).reshape(GPC, T)
    return (y, pred)
